# revision 6
# baseline (speedup 1.0000x reference)
"""Trainium2 Bass kernel for nn_Attn_48206712930921.

Computes softmax(mask(einsum('bsh,oh->bso', outputs, W) + b) @ weight_vec)
== softmax over s of energies[b,s], where algebraically

    energies[b,s] = outputs[b,s,:] . (W^T @ weight_vec) + (weight_vec . b)

so the [B,S,H]x[H,H] projection collapses to a length-H dot product per
(b,s) row.  The kernel is therefore memory bound: each of the 8 cores
streams its 64MB shard of `outputs` through SBUF once; the vector engine
forms x*v products while the scalar engine row-reduces them via a
Copy-activation with accumulate, and a per-batch masked softmax finishes
on-chip.

Sharding: data parallel over batch (8 batches per core), per the hint.
"""

import numpy as np

import concourse.bacc as bacc
import concourse.bass as bass
import concourse.tile as tile
from concourse import mybir
from concourse.bass_utils import run_bass_kernel_spmd

B, S, H = 64, 2048, 1024
NCORES = 8
BPC = B // NCORES          # batches per core
ROWS = BPC * S             # rows (b,s pairs) per core
CHUNK = 128                # rows per DVE op (one SBUF partition each)
NCHUNK = ROWS // CHUNK     # 128 chunks per core
GROUP = 8                  # chunks fetched per DMA (4 MiB transfers)
CPB = S // CHUNK           # chunks per batch (16)
GPB = CPB // GROUP         # DMA groups per batch (2)
# chunks (by index within a DMA group) whose multiply runs on GpSimd with
# the row-reduce on DVE; the rest multiply on DVE and reduce on ScalarE.
# Balances the three engines under the DMA roofline.
POOL_CHUNKS = (3, 7)
NEG = -1.0e10

f32 = mybir.dt.float32

_cached = {}


def _build():
    nc = bacc.Bacc("TRN2", target_bir_lowering=False, debug=False,
                   num_devices=NCORES)

    x = nc.dram_tensor("x", [ROWS, H], f32, kind="ExternalInput")
    v = nc.dram_tensor("v", [H], f32, kind="ExternalInput")
    maskb = nc.dram_tensor("maskb", [BPC, S], f32, kind="ExternalInput")
    ident = nc.dram_tensor("ident", [128, 128], f32, kind="ExternalInput")
    out = nc.dram_tensor("out", [BPC, S], f32, kind="ExternalOutput")

    xv = x.ap().rearrange("(n p) h -> n p h", p=CHUNK)  # [NCHUNK, 128, H]

    with tile.TileContext(nc) as tc:
        with tc.tile_pool(name="singles", bufs=1) as singles, \
             tc.tile_pool(name="xp", bufs=3) as xp, \
             tc.tile_pool(name="prodp", bufs=4) as prodp, \
             tc.tile_pool(name="ep", bufs=2) as ep, \
             tc.tile_pool(name="sp", bufs=2) as sp, \
             tc.tile_pool(name="pp", bufs=2, space="PSUM") as pp, \
             tc.tile_pool(name="dumpp", bufs=1, space="PSUM") as dumpp:

            # v replicated across all 128 partitions via 0-stride DMA
            vb = singles.tile([128, H], f32)
            v_ap = v.ap()
            v_bcast = bass.AP(tensor=v_ap.tensor, offset=v_ap.offset,
                              ap=[[0, 128]] + list(v_ap.ap))
            nc.gpsimd.dma_start(out=vb, in_=v_bcast)

            maskt = singles.tile([BPC, S], f32)
            nc.sync.dma_start(out=maskt, in_=maskb[:, :])
            identt = singles.tile([128, 128], f32)
            nc.sync.dma_start(out=identt, in_=ident[:, :])

            # scratch target for the scalar engine's (unused) copy output
            dump = dumpp.tile([128, H], f32)
            # energies for all 8 batches of this core, one batch per partition
            e_all = singles.tile([BPC, S], f32)

            for bi in range(BPC):
                ebuf = ep.tile([128, CPB], f32)
                for gg in range(GPB):
                    g = bi * GPB + gg
                    xt = xp.tile([128, GROUP, H], f32)
                    src = xv[g * GROUP:(g + 1) * GROUP].rearrange(
                        "n p h -> p n h")
                    nc.sync.dma_start(out=xt, in_=src)
                    for n in range(GROUP):
                        col = gg * GROUP + n
                        prod = prodp.tile([128, H], f32)
                        if n in POOL_CHUNKS:
                            nc.gpsimd.tensor_mul(prod, xt[:, n, :], vb)
                            nc.vector.reduce_sum(ebuf[:, col:col + 1], prod,
                                                 axis=mybir.AxisListType.X)
                        else:
                            nc.vector.tensor_mul(prod, xt[:, n, :], vb)
                            # row-sum on ScalarE: accum_out of a Copy
                            nc.scalar.activation(
                                out=dump, in_=prod,
                                func=mybir.ActivationFunctionType.Copy,
                                accum_out=ebuf[:, col:col + 1])
                # reshape this batch's energies [128, 16] -> [1, 2048] row:
                # TensorE transpose puts s = col*128+p in partition-major
                # order, then an SBUF->SBUF DMA collapses it into e_all[bi].
                pt = pp.tile([CPB, 128], f32)
                nc.tensor.transpose(pt, ebuf, identt)
                et = sp.tile([CPB, 128], f32)
                nc.scalar.copy(et, pt)
                nc.sync.dma_start(out=e_all[bi:bi + 1, :], in_=et)

            # masked softmax along s for all 8 batches at once
            em = sp.tile([BPC, S], f32)
            nc.vector.tensor_add(em, e_all, maskt)
            m = sp.tile([BPC, 1], f32)
            nc.vector.reduce_max(m, em, axis=mybir.AxisListType.X)
            negm = sp.tile([BPC, 1], f32)
            nc.vector.tensor_scalar_mul(negm, m, -1.0)
            expa = sp.tile([BPC, S], f32)
            sume = sp.tile([BPC, 1], f32)
            nc.scalar.activation(out=expa, in_=em,
                                 func=mybir.ActivationFunctionType.Exp,
                                 bias=negm, scale=1.0, accum_out=sume)
            rinv = sp.tile([BPC, 1], f32)
            nc.vector.reciprocal(rinv, sume)
            outt = sp.tile([BPC, S], f32)
            nc.vector.tensor_scalar_mul(outt, expa, rinv)
            nc.sync.dma_start(out=out[:, :], in_=outt)

    nc.compile()
    return nc


def _get_nc():
    if "nc" not in _cached:
        _cached["nc"] = _build()
    return _cached["nc"]


def _in_maps(outputs, text_lens, W, b, weight_vec):
    v = (W.astype(np.float64).T @ weight_vec.astype(np.float64)).astype(
        np.float32)
    c = np.float32(weight_vec.astype(np.float64) @ b.astype(np.float64))
    pos = np.arange(S)[None, :]
    # energies = x.v + c for s < len, ~NEG for s >= len (exp underflows to 0
    # exactly, matching the reference's hard -1e10 fill after softmax)
    mask_full = np.where(pos < np.asarray(text_lens)[:, None], c,
                         np.float32(NEG)).astype(np.float32)  # [B, S]
    ident = np.eye(128, dtype=np.float32)
    maps = []
    for k in range(NCORES):
        xk = np.ascontiguousarray(
            outputs[k * BPC:(k + 1) * BPC].reshape(ROWS, H))
        mk = np.ascontiguousarray(mask_full[k * BPC:(k + 1) * BPC])
        maps.append({"x": xk, "v": v, "maskb": mk, "ident": ident})
    return maps


def _gather(res):
    return np.concatenate([res.results[k]["out"] for k in range(NCORES)],
                          axis=0)


def kernel(outputs, text_lens, W, b, weight_vec):
    nc = _get_nc()
    maps = _in_maps(outputs, text_lens, W, b, weight_vec)
    res = run_bass_kernel_spmd(nc, maps, list(range(NCORES)))
    return _gather(res)


def kernel_traced(outputs, text_lens, W, b, weight_vec, **trace_kwargs):
    """Like kernel() but profiles the run; returns (output, BassKernelResults)."""
    nc = _get_nc()
    maps = _in_maps(outputs, text_lens, W, b, weight_vec)
    res = run_bass_kernel_spmd(nc, maps, list(range(NCORES)), trace=True,
                               **trace_kwargs)
    return _gather(res), res


# revision 8
# speedup vs baseline: 1.1699x; 1.1699x over previous
"""Trainium2 Bass kernel for nn_Attn_48206712930921.

Computes softmax(mask(einsum('bsh,oh->bso', outputs, W) + b) @ weight_vec)
== softmax over s of energies[b,s], where algebraically

    energies[b,s] = outputs[b,s,:] . (W^T @ weight_vec) + (weight_vec . b)

so the [B,S,H]x[H,H] projection collapses to a length-H dot product per
(b,s) row.  The kernel is therefore memory bound: each of the 8 cores
streams its 64MB shard of `outputs` through SBUF once; the vector engine
forms x*v products while the scalar engine row-reduces them via a
Copy-activation with accumulate, and a per-batch masked softmax finishes
on-chip.

Sharding: data parallel over batch (8 batches per core), per the hint.
"""

import numpy as np

import concourse.bacc as bacc
import concourse.bass as bass
import concourse.tile as tile
from concourse import mybir
from concourse.bass_utils import run_bass_kernel_spmd

B, S, H = 64, 2048, 1024
NCORES = 8
BPC = B // NCORES          # batches per core
ROWS = BPC * S             # rows (b,s pairs) per core
CHUNK = 128                # rows per DVE op (one SBUF partition each)
NCHUNK = ROWS // CHUNK     # 128 chunks per core
GROUP = 8                  # chunks fetched per DMA (4 MiB transfers)
CPB = S // CHUNK           # chunks per batch (16)
GPB = CPB // GROUP         # DMA groups per batch (2)
# chunks (by index within a DMA group) whose row-reduce runs on DVE via
# tensor_reduce; the rest reduce on ScalarE via Copy-activation accumulate.
# Balances DVE and ScalarE under the DMA roofline.  (GpSimd is useless here:
# concurrent Pool elemwise degrades DVE tensor_tensor ~1.6x via SBUF port
# sharing, measured 1228 -> 1931 ns.)
DVE_REDUCE_CHUNKS = (3,)
NEG = -1.0e10

f32 = mybir.dt.float32

_cached = {}


def _build():
    nc = bacc.Bacc("TRN2", target_bir_lowering=False, debug=False,
                   num_devices=NCORES)

    x = nc.dram_tensor("x", [ROWS, H], f32, kind="ExternalInput")
    v = nc.dram_tensor("v", [H], f32, kind="ExternalInput")
    maskb = nc.dram_tensor("maskb", [BPC, S], f32, kind="ExternalInput")
    ident = nc.dram_tensor("ident", [128, 128], f32, kind="ExternalInput")
    out = nc.dram_tensor("out", [BPC, S], f32, kind="ExternalOutput")

    xv = x.ap().rearrange("(n p) h -> n p h", p=CHUNK)  # [NCHUNK, 128, H]

    with tile.TileContext(nc) as tc:
        with tc.tile_pool(name="singles", bufs=1) as singles, \
             tc.tile_pool(name="xp", bufs=3) as xp, \
             tc.tile_pool(name="prodp", bufs=4) as prodp, \
             tc.tile_pool(name="ep", bufs=2) as ep, \
             tc.tile_pool(name="sp", bufs=2) as sp, \
             tc.tile_pool(name="pp", bufs=2, space="PSUM") as pp, \
             tc.tile_pool(name="dumpp", bufs=1, space="PSUM") as dumpp:

            # v replicated across all 128 partitions via 0-stride DMA
            vb = singles.tile([128, H], f32)
            v_ap = v.ap()
            v_bcast = bass.AP(tensor=v_ap.tensor, offset=v_ap.offset,
                              ap=[[0, 128]] + list(v_ap.ap))
            nc.gpsimd.dma_start(out=vb, in_=v_bcast)

            maskt = singles.tile([BPC, S], f32)
            nc.sync.dma_start(out=maskt, in_=maskb[:, :])
            identt = singles.tile([128, 128], f32)
            nc.sync.dma_start(out=identt, in_=ident[:, :])

            # scratch target for the scalar engine's (unused) copy output
            dump = dumpp.tile([128, H], f32)
            # energies for all 8 batches of this core, one batch per partition
            e_all = singles.tile([BPC, S], f32)

            for bi in range(BPC):
                ebuf = ep.tile([128, CPB], f32)
                for gg in range(GPB):
                    g = bi * GPB + gg
                    xt = xp.tile([128, GROUP, H], f32)
                    src = xv[g * GROUP:(g + 1) * GROUP].rearrange(
                        "n p h -> p n h")
                    nc.sync.dma_start(out=xt, in_=src)
                    for n in range(GROUP):
                        col = gg * GROUP + n
                        prod = prodp.tile([128, H], f32)
                        nc.vector.tensor_mul(prod, xt[:, n, :], vb)
                        if n in DVE_REDUCE_CHUNKS:
                            nc.vector.reduce_sum(ebuf[:, col:col + 1], prod,
                                                 axis=mybir.AxisListType.X)
                        else:
                            # row-sum on ScalarE: accum_out of a Copy
                            nc.scalar.activation(
                                out=dump, in_=prod,
                                func=mybir.ActivationFunctionType.Copy,
                                accum_out=ebuf[:, col:col + 1])
                # reshape this batch's energies [128, 16] -> [1, 2048] row:
                # TensorE transpose puts s = col*128+p in partition-major
                # order, then an SBUF->SBUF DMA collapses it into e_all[bi].
                pt = pp.tile([CPB, 128], f32)
                nc.tensor.transpose(pt, ebuf, identt)
                et = sp.tile([CPB, 128], f32)
                nc.scalar.copy(et, pt)
                nc.sync.dma_start(out=e_all[bi:bi + 1, :], in_=et)

            # masked softmax along s for all 8 batches at once
            em = sp.tile([BPC, S], f32)
            nc.vector.tensor_add(em, e_all, maskt)
            m = sp.tile([BPC, 1], f32)
            nc.vector.reduce_max(m, em, axis=mybir.AxisListType.X)
            negm = sp.tile([BPC, 1], f32)
            nc.vector.tensor_scalar_mul(negm, m, -1.0)
            expa = sp.tile([BPC, S], f32)
            sume = sp.tile([BPC, 1], f32)
            nc.scalar.activation(out=expa, in_=em,
                                 func=mybir.ActivationFunctionType.Exp,
                                 bias=negm, scale=1.0, accum_out=sume)
            rinv = sp.tile([BPC, 1], f32)
            nc.vector.reciprocal(rinv, sume)
            outt = sp.tile([BPC, S], f32)
            nc.vector.tensor_scalar_mul(outt, expa, rinv)
            nc.sync.dma_start(out=out[:, :], in_=outt)

    nc.compile()
    return nc


def _get_nc():
    if "nc" not in _cached:
        _cached["nc"] = _build()
    return _cached["nc"]


def _in_maps(outputs, text_lens, W, b, weight_vec):
    v = (W.astype(np.float64).T @ weight_vec.astype(np.float64)).astype(
        np.float32)
    c = np.float32(weight_vec.astype(np.float64) @ b.astype(np.float64))
    pos = np.arange(S)[None, :]
    # energies = x.v + c for s < len, ~NEG for s >= len (exp underflows to 0
    # exactly, matching the reference's hard -1e10 fill after softmax)
    mask_full = np.where(pos < np.asarray(text_lens)[:, None], c,
                         np.float32(NEG)).astype(np.float32)  # [B, S]
    ident = np.eye(128, dtype=np.float32)
    maps = []
    for k in range(NCORES):
        xk = np.ascontiguousarray(
            outputs[k * BPC:(k + 1) * BPC].reshape(ROWS, H))
        mk = np.ascontiguousarray(mask_full[k * BPC:(k + 1) * BPC])
        maps.append({"x": xk, "v": v, "maskb": mk, "ident": ident})
    return maps


def _gather(res):
    return np.concatenate([res.results[k]["out"] for k in range(NCORES)],
                          axis=0)


def kernel(outputs, text_lens, W, b, weight_vec):
    nc = _get_nc()
    maps = _in_maps(outputs, text_lens, W, b, weight_vec)
    res = run_bass_kernel_spmd(nc, maps, list(range(NCORES)))
    return _gather(res)


def kernel_traced(outputs, text_lens, W, b, weight_vec, **trace_kwargs):
    """Like kernel() but profiles the run; returns (output, BassKernelResults)."""
    nc = _get_nc()
    maps = _in_maps(outputs, text_lens, W, b, weight_vec)
    res = run_bass_kernel_spmd(nc, maps, list(range(NCORES)), trace=True,
                               **trace_kwargs)
    return _gather(res), res


# revision 9
# speedup vs baseline: 1.3628x; 1.1648x over previous
"""Trainium2 Bass kernel for nn_Attn_48206712930921.

Computes softmax(mask(einsum('bsh,oh->bso', outputs, W) + b) @ weight_vec)
== softmax over s of energies[b,s], where algebraically

    energies[b,s] = outputs[b,s,:] . (W^T @ weight_vec) + (weight_vec . b)

so the [B,S,H]x[H,H] projection collapses to a length-H dot product per
(b,s) row.  The kernel is therefore memory bound: each of the 8 cores
streams its 64MB shard of `outputs` through SBUF once; the vector engine
forms x*v products while the scalar engine row-reduces them via a
Copy-activation with accumulate, and a per-batch masked softmax finishes
on-chip.

Sharding: data parallel over batch (8 batches per core), per the hint.
"""

import numpy as np

import concourse.bacc as bacc
import concourse.bass as bass
import concourse.tile as tile
from concourse import mybir
from concourse.bass_utils import run_bass_kernel_spmd

B, S, H = 64, 2048, 1024
NCORES = 8
BPC = B // NCORES          # batches per core
ROWS = BPC * S             # rows (b,s pairs) per core
CHUNK = 128                # rows per DVE op (one SBUF partition each)
NCHUNK = ROWS // CHUNK     # 128 chunks per core
GROUP = 4                  # chunks fetched per DMA (2 MiB transfers)
CPB = S // CHUNK           # chunks per batch (16)
GPB = CPB // GROUP         # DMA groups per batch (4)
NEG = -1.0e10

f32 = mybir.dt.float32

_cached = {}


def _build():
    nc = bacc.Bacc("TRN2", target_bir_lowering=False, debug=False,
                   num_devices=NCORES)

    x = nc.dram_tensor("x", [ROWS, H], f32, kind="ExternalInput")
    v = nc.dram_tensor("v", [H], f32, kind="ExternalInput")
    maskb = nc.dram_tensor("maskb", [BPC, S], f32, kind="ExternalInput")
    ident = nc.dram_tensor("ident", [128, 128], f32, kind="ExternalInput")
    out = nc.dram_tensor("out", [BPC, S], f32, kind="ExternalOutput")

    xv = x.ap().rearrange("(n p) h -> n p h", p=CHUNK)  # [NCHUNK, 128, H]

    with tile.TileContext(nc) as tc:
        with tc.tile_pool(name="singles", bufs=1) as singles, \
             tc.tile_pool(name="xp", bufs=3) as xp, \
             tc.tile_pool(name="prodp", bufs=4) as prodp, \
             tc.tile_pool(name="ep", bufs=2) as ep, \
             tc.tile_pool(name="sp", bufs=2) as sp, \
             tc.tile_pool(name="pp", bufs=2, space="PSUM") as pp, \
             tc.tile_pool(name="dumpp", bufs=1, space="PSUM") as dumpp:

            # v replicated across all 128 partitions and GROUP chunk slots
            # via 0-stride DMA, so one wide DVE multiply covers a whole group
            vb = singles.tile([128, GROUP, H], f32)
            v_ap = v.ap()
            v_bcast = bass.AP(tensor=v_ap.tensor, offset=v_ap.offset,
                              ap=[[0, 128], [0, GROUP]] + list(v_ap.ap))
            nc.gpsimd.dma_start(out=vb, in_=v_bcast)

            maskt = singles.tile([BPC, S], f32)
            nc.sync.dma_start(out=maskt, in_=maskb[:, :])
            identt = singles.tile([128, 128], f32)
            nc.sync.dma_start(out=identt, in_=ident[:, :])

            # scratch target for the scalar engine's (unused) copy output
            dump = dumpp.tile([128, H], f32)
            # energies for all 8 batches of this core, one batch per partition
            e_all = singles.tile([BPC, S], f32)

            for bi in range(BPC):
                ebuf = ep.tile([128, CPB], f32)
                for gg in range(GPB):
                    g = bi * GPB + gg
                    xt = xp.tile([128, GROUP, H], f32)
                    src = xv[g * GROUP:(g + 1) * GROUP].rearrange(
                        "n p h -> p n h")
                    nc.sync.dma_start(out=xt, in_=src)
                    # one wide multiply for the whole group (amortizes the
                    # ~151-cycle DVE op overhead and per-op semaphores)
                    prod = prodp.tile([128, GROUP, H], f32)
                    nc.vector.tensor_mul(prod, xt, vb)
                    for n in range(GROUP):
                        col = gg * GROUP + n
                        if n == GROUP - 1 and (g % 2 == 1):
                            # every other group: last chunk reduces on DVE to
                            # offload ScalarE (keeps both under the DMA bound)
                            nc.vector.reduce_sum(ebuf[:, col:col + 1],
                                                 prod[:, n, :],
                                                 axis=mybir.AxisListType.X)
                        else:
                            # row-sum on ScalarE: accum_out of a Copy
                            nc.scalar.activation(
                                out=dump, in_=prod[:, n, :],
                                func=mybir.ActivationFunctionType.Copy,
                                accum_out=ebuf[:, col:col + 1])
                # reshape this batch's energies [128, 16] -> [1, 2048] row:
                # TensorE transpose puts s = col*128+p in partition-major
                # order, then an SBUF->SBUF DMA collapses it into e_all[bi].
                pt = pp.tile([CPB, 128], f32)
                nc.tensor.transpose(pt, ebuf, identt)
                et = sp.tile([CPB, 128], f32)
                nc.vector.tensor_copy(et, pt)
                nc.sync.dma_start(out=e_all[bi:bi + 1, :], in_=et)

            # masked softmax along s for all 8 batches at once
            em = singles.tile([BPC, S], f32)
            nc.vector.tensor_add(em, e_all, maskt)
            m = sp.tile([BPC, 1], f32)
            nc.vector.reduce_max(m, em, axis=mybir.AxisListType.X)
            negm = sp.tile([BPC, 1], f32)
            nc.vector.tensor_scalar_mul(negm, m, -1.0)
            expa = singles.tile([BPC, S], f32)
            sume = sp.tile([BPC, 1], f32)
            nc.scalar.activation(out=expa, in_=em,
                                 func=mybir.ActivationFunctionType.Exp,
                                 bias=negm, scale=1.0, accum_out=sume)
            rinv = sp.tile([BPC, 1], f32)
            nc.vector.reciprocal(rinv, sume)
            outt = singles.tile([BPC, S], f32)
            nc.vector.tensor_scalar_mul(outt, expa, rinv)
            nc.sync.dma_start(out=out[:, :], in_=outt)

    nc.compile()
    return nc


def _get_nc():
    if "nc" not in _cached:
        _cached["nc"] = _build()
    return _cached["nc"]


def _in_maps(outputs, text_lens, W, b, weight_vec):
    v = (W.astype(np.float64).T @ weight_vec.astype(np.float64)).astype(
        np.float32)
    c = np.float32(weight_vec.astype(np.float64) @ b.astype(np.float64))
    pos = np.arange(S)[None, :]
    # energies = x.v + c for s < len, ~NEG for s >= len (exp underflows to 0
    # exactly, matching the reference's hard -1e10 fill after softmax)
    mask_full = np.where(pos < np.asarray(text_lens)[:, None], c,
                         np.float32(NEG)).astype(np.float32)  # [B, S]
    ident = np.eye(128, dtype=np.float32)
    maps = []
    for k in range(NCORES):
        xk = np.ascontiguousarray(
            outputs[k * BPC:(k + 1) * BPC].reshape(ROWS, H))
        mk = np.ascontiguousarray(mask_full[k * BPC:(k + 1) * BPC])
        maps.append({"x": xk, "v": v, "maskb": mk, "ident": ident})
    return maps


def _gather(res):
    return np.concatenate([res.results[k]["out"] for k in range(NCORES)],
                          axis=0)


def kernel(outputs, text_lens, W, b, weight_vec):
    nc = _get_nc()
    maps = _in_maps(outputs, text_lens, W, b, weight_vec)
    res = run_bass_kernel_spmd(nc, maps, list(range(NCORES)))
    return _gather(res)


def kernel_traced(outputs, text_lens, W, b, weight_vec, **trace_kwargs):
    """Like kernel() but profiles the run; returns (output, BassKernelResults)."""
    nc = _get_nc()
    maps = _in_maps(outputs, text_lens, W, b, weight_vec)
    res = run_bass_kernel_spmd(nc, maps, list(range(NCORES)), trace=True,
                               **trace_kwargs)
    return _gather(res), res
